# revision 37
# baseline (speedup 1.0000x reference)
"""MiniMax-M2 sparse MoE block on 8 Trainium2 NeuronCores.

Strategy (expert-parallel):
  - Host: router (1024x16 gating matmul + softmax + biased top-2), token
    dispatch (gather tokens per expert, pad to capacity C), weight layout
    prep, and the final weighted combine/scatter.  All of this is tiny
    data-movement next to the expert MLPs.
  - Device: each of the 8 cores owns 2 of the 16 experts and runs the
    SwiGLU MLP (w1/w3 -> silu*mul -> w2) for its experts' tokens.
    Everything is computed in transposed token layout so both weight
    matrices are used as the matmul stationary operand in natural layout:
        h1T[I,C] = sum_k w1[k,I].T @ xT[k,C]      (k = 128-row H chunks)
        heT      = silu(h1T) * h3T
        yT[H,C]  = sum_i w2b[i,H].T @ heT[i,C]    (i = 128-row I chunks)
    No transposes anywhere on device; no collectives (host combines).

Shapes are hardcoded per the problem spec: T=1024, H=2048, I=1024,
E=16 experts, top-2, fp32 I/O.
"""

import os

import numpy as np
import ml_dtypes

T, H, I, E, TOPK = 1024, 2048, 1024, 16, 2
N_CORES = 8
E_LOC = E // N_CORES  # experts per core
NB = 256              # tokens per matmul N-chunk (PSUM-bank friendly)
P = 128               # partition size

# precision of the device matmuls: "bf16" | "f32r" | "f32"
PREC = os.environ.get("MOE_PREC", "bf16")
TRACE = os.environ.get("MOE_TRACE", "0") == "1"
# phase order: "aabb" = A(e0),A(e1),B(e0),B(e1); "abab" = per-expert A,B
AABB = os.environ.get("MOE_AABB", "1") == "1"

LAST_RESULTS = None  # BassKernelResults of the last run (for test harness)
_RUN_IDX = 0

_BUILD_CACHE: dict = {}


def _np_wdtype(prec: str):
    if prec == "bf16":
        return ml_dtypes.bfloat16
    return np.float32  # f32r is stored as raw fp32 bytes


def _build(C: int, prec: str):
    """Build + lower the per-core Bass program (same SPMD program on all
    cores; per-core data differs via in_maps)."""
    key = (C, prec, AABB)
    if key in _BUILD_CACHE:
        return _BUILD_CACHE[key]

    import concourse.bacc as bacc
    import concourse.tile as tile
    import concourse.mybir as mybir
    from concourse.bass import ts, ds

    dt_w = {
        "bf16": mybir.dt.bfloat16,
        "f32r": mybir.dt.float32r,
        "f32": mybir.dt.float32,
    }[prec]
    f32 = mybir.dt.float32

    KH = H // P   # 16 contraction chunks over H
    MI = I // P   # 8 output blocks over I (stage A), contraction chunks (stage B)
    MH = H // P   # 16 output blocks over H (stage B)
    # token chunking: single chunk of width C when it fits a PSUM bank
    if C <= 512:
        NC, nb = 1, C
    else:
        NC, nb = C // NB, NB

    nc = bacc.Bacc("TRN2", target_bir_lowering=False, debug=False,
                   num_devices=N_CORES)

    # xT pre-tiled on host: xT[e, p, kb*C+c] = x_gathered[e][kb*128+p, c]
    xT = nc.dram_tensor("xT", [E_LOC, P, KH * C], dt_w, kind="ExternalInput")
    w1 = nc.dram_tensor("w1", [E_LOC, H, I], dt_w, kind="ExternalInput")
    w3 = nc.dram_tensor("w3", [E_LOC, H, I], dt_w, kind="ExternalInput")
    # w2 pre-blocked on host: w2b[e, hb, r, ib*128+c] = w2[e, ib*128+r, hb*128+c]
    w2b = nc.dram_tensor("w2b", [E_LOC, MH, P, I], dt_w, kind="ExternalInput")
    # yT tiled: yT[e, p, hb*C+c] = y_e[hb*128+p, c]; host un-tiles.
    # bf16 output: the combine weights/sum run in fp64 on host, and the
    # expert outputs already carry bf16-compute noise, so bf16 here only
    # adds one rounding step while halving output DMA bytes.
    yT = nc.dram_tensor("yT", [E_LOC, P, MH * C], mybir.dt.bfloat16,
                        kind="ExternalOutput")

    OG = 4            # output DMA groups per expert
    GH = MH // OG
    XS = 4            # xbig load split (smaller first transfer = lower latency)

    # w1/w3 chunks are consumed once each (kb-outer loop) so they stream;
    # bufs only needs to cover the DMA pipeline depth.  The C>512 fallback
    # reuses chunks across cb passes, so they must stay resident there.
    WP_BUFS = 16 if NC == 1 else 2 * KH + 4
    W2P_BUFS = 2 * MH if NC == 1 else 6
    YP_BUFS = 2 if NC == 1 else 1

    with tile.TileContext(nc) as tc:
        with (
            tc.tile_pool(name="xp", bufs=E_LOC) as xp,
            tc.tile_pool(name="wp", bufs=WP_BUFS) as wp,
            tc.tile_pool(name="w2p", bufs=W2P_BUFS) as w2p,
            tc.tile_pool(name="hp", bufs=2 * MI * NC) as hp,
            tc.tile_pool(name="sp", bufs=3) as sp,
            tc.tile_pool(name="yp", bufs=YP_BUFS) as yp,
            tc.tile_pool(name="pk", bufs=MI, space="PSUM") as pk,
        ):
            # All load-DMAs are emitted up front on the Sync HWDGE queue in
            # exact consumption order (a single stream saturates the fabric;
            # a second queue just steals its bandwidth).  Output DMAs ride
            # the otherwise-idle GpSimd SWDGE queue.
            # Phase order is A(e0), A(e1), B(e0), B(e1): the PE then consumes
            # the single DMA stream strictly in queue order with no
            # pipeline-refill bubble at phase transitions, and each expert's
            # ACT/DVE nonlinearity chain hides under the next phase's MMs.
            xbig = []
            for e in range(E_LOC):
                t = xp.tile([P, KH * C], dt_w, tag="xt", name=f"xt{e}")
                xbig.append(t)
            step = (KH * C) // XS

            def load_xbig(e, q0, q1):
                # token loads ride the otherwise-idle GpSimd SWDGE queue,
                # keeping the sync HWDGE stream pure weight traffic
                for q in range(q0, q1):
                    nc.gpsimd.dma_start(xbig[e][:, ds(q * step, step)],
                                        xT[e, :, ds(q * step, step)])

            w1t_all, w3t_all, w2t_all = [[], []], [[], []], [[], []]

            def load_w13(e, kb):
                a = wp.tile([P, I], dt_w, tag="w1", name=f"w1_{e}_{kb}")
                nc.sync.dma_start(a[:], w1[e, ts(kb, P), :])
                w1t_all[e].append(a)
                b = wp.tile([P, I], dt_w, tag="w3", name=f"w3_{e}_{kb}")
                nc.sync.dma_start(b[:], w3[e, ts(kb, P), :])
                w3t_all[e].append(b)

            def load_w2(e):
                w2t = []
                for hb in range(MH):
                    wt = w2p.tile([P, I], dt_w, tag="w2", name=f"w2_{e}_{hb}")
                    nc.sync.dma_start(wt[:], w2b[e, hb, :, :])
                    w2t.append(wt)
                w2t_all[e] = w2t

            for e in range(E_LOC):
                load_xbig(e, 0, XS)
            if AABB:
                for e in range(E_LOC):
                    for kb in range(KH):
                        load_w13(e, kb)
                for e in range(E_LOC):
                    load_w2(e)
            else:
                for e in range(E_LOC):
                    for kb in range(KH):
                        load_w13(e, kb)
                    load_w2(e)

            het_all = [None] * E_LOC

            def stage_A(e):
                w1t, w3t = w1t_all[e], w3t_all[e]

                # Stage A, kb-outer: all MI h1/h3 blocks accumulate at once,
                # so the PE consumes each weight chunk the moment it lands.
                # h1 and h3 for one ib share a single PSUM bank: p13[:, :nb]
                # is h1, p13[:, nb:] is h3.
                het = [[None] * MI for _ in range(NC)]
                for cb in range(NC):
                    p13 = [pk.tile([P, 2 * nb], f32, tag="pk",
                                   name=f"p13_{e}_{cb}_{ib}")
                           for ib in range(MI)]
                    # One accumulation group per bank: start=True only on the
                    # bank's first matmul (h1,kb=0) -- it clears has_written
                    # for the whole bank; h3's kb=0 then lands by per-element
                    # overwrite-where-unwritten.  stop on the bank's last MM.
                    for kb in range(KH - 1):
                        rhs = xbig[e][:, ds(kb * C + cb * nb, nb)]
                        for ib in range(MI):
                            nc.tensor.matmul(
                                p13[ib][:, ds(0, nb)], w1t[kb][:, ts(ib, P)],
                                rhs, start=(kb == 0), stop=False)
                        for ib in range(MI):
                            nc.tensor.matmul(
                                p13[ib][:, ds(nb, nb)], w3t[kb][:, ts(ib, P)],
                                rhs, start=False, stop=False)
                    # Last chunk pairwise per-ib so each bank closes (and its
                    # silu chain + PSUM slot release starts) as early as
                    # possible instead of all at once.
                    kb = KH - 1
                    rhs = xbig[e][:, ds(kb * C + cb * nb, nb)]
                    for ib in range(MI):
                        nc.tensor.matmul(
                            p13[ib][:, ds(0, nb)], w1t[kb][:, ts(ib, P)],
                            rhs, start=False, stop=False)
                        nc.tensor.matmul(
                            p13[ib][:, ds(nb, nb)], w3t[kb][:, ts(ib, P)],
                            rhs, start=False, stop=True)
                        # silu(h1)=h1*sigmoid(h1); CoreSim lacks a Silu LUT
                        s = sp.tile([P, nb], f32, tag="s", name=f"s_{e}_{cb}_{ib}")
                        nc.scalar.activation(
                            s[:], p13[ib][:, ds(0, nb)],
                            mybir.ActivationFunctionType.Sigmoid)
                        u = sp.tile([P, nb], f32, tag="u", name=f"u_{e}_{cb}_{ib}")
                        nc.vector.tensor_mul(u[:], s[:], p13[ib][:, ds(0, nb)])
                        h = hp.tile([P, nb], dt_w, tag="he", name=f"he_{e}_{cb}_{ib}")
                        nc.vector.tensor_mul(h[:], u[:], p13[ib][:, ds(nb, nb)])
                        het[cb][ib] = h
                het_all[e] = het

            def stage_B(e):
                het = het_all[e]
                # Stage B: yT[hb] = sum_ib w2b.T @ heT; py tiles share the
                # stage-A PSUM slots (single pool) so both experts fit.
                yst = yp.tile([P, MH * C], mybir.dt.bfloat16, tag="yst",
                              name=f"yst_{e}")
                for hb in range(MH):
                    wt = w2t_all[e][hb]
                    for cb in range(NC):
                        py = pk.tile([P, nb], f32, tag="pk",
                                     name=f"py_{e}_{hb}_{cb}")
                        for ib in range(MI):
                            nc.tensor.matmul(
                                py[:], wt[:, ts(ib, P)], het[cb][ib][:],
                                start=(ib == 0), stop=(ib == MI - 1))
                        nc.vector.tensor_copy(
                            yst[:, ds(hb * C + cb * nb, nb)], py[:])
                    if (hb + 1) % GH == 0:
                        g0 = (hb + 1 - GH) * C
                        nc.gpsimd.dma_start(
                            yT[e, :, ds(g0, GH * C)], yst[:, ds(g0, GH * C)])

            if AABB:
                for e in range(E_LOC):
                    stage_A(e)
                for e in range(E_LOC):
                    stage_B(e)
            else:
                for e in range(E_LOC):
                    stage_A(e)
                    stage_B(e)

    nc.compile()
    _BUILD_CACHE[key] = nc
    return nc


def _route(x: np.ndarray, gate_w: np.ndarray, bias: np.ndarray):
    """Reference-equivalent router, done in fp64 for tie stability.
    Returns per-expert token index lists and combine weights."""
    logits = x.astype(np.float64) @ gate_w.astype(np.float64).T      # [T, E]
    m = logits.max(axis=1, keepdims=True)
    p = np.exp(logits - m)
    scores = p / p.sum(axis=1, keepdims=True)                        # [T, E]
    biased = scores + bias.astype(np.float64)[None, :]
    # top-2, ties to lower index (matches jax.lax.top_k)
    idx = np.argsort(-biased, axis=1, kind="stable")[:, :TOPK]       # [T, 2]
    tw = np.take_along_axis(scores, idx, axis=1)
    tw = tw / tw.sum(axis=1, keepdims=True)                          # [T, 2]

    flat_e = idx.ravel()
    flat_t = np.repeat(np.arange(T), TOPK)
    flat_w = tw.ravel()
    order = np.argsort(flat_e, kind="stable")
    fe, ft, fw = flat_e[order], flat_t[order], flat_w[order]
    starts = np.searchsorted(fe, np.arange(E + 1))
    tok = [ft[starts[e]:starts[e + 1]] for e in range(E)]
    wgt = [fw[starts[e]:starts[e + 1]] for e in range(E)]
    return tok, wgt


def kernel(hidden_states, gate_w, bias, w1, w3, w2):
    global LAST_RESULTS
    from concourse.bass_utils import run_bass_kernel_spmd

    x = np.asarray(hidden_states, dtype=np.float32)
    gate_w = np.asarray(gate_w, dtype=np.float32)
    bias = np.asarray(bias, dtype=np.float32)
    w1 = np.asarray(w1, dtype=np.float32)
    w3 = np.asarray(w3, dtype=np.float32)
    w2 = np.asarray(w2, dtype=np.float32)

    tok, wgt = _route(x, gate_w, bias)
    max_count = max(len(t) for t in tok)
    C = max(128, 32 * ((max_count + 31) // 32))
    if C > 512:  # rare: very imbalanced routing; fall back to 256-chunks
        C = NB * ((C + NB - 1) // NB)

    wdt = _np_wdtype(PREC)
    MI, MH = I // P, H // P

    # Gather + transpose tokens per expert: xT_all[e] = x[tok[e]].T padded,
    # then pre-tile to [E, P, KH*C] for single-DMA loads.
    KH = H // P
    xT_all = np.zeros((E, H, C), dtype=wdt)
    xt_f32 = x.T  # [H, T]
    for e in range(E):
        n = len(tok[e])
        if n:
            xT_all[e, :, :n] = xt_f32[:, tok[e]].astype(wdt)
    xT_tiled = np.ascontiguousarray(
        xT_all.reshape(E, KH, P, C).transpose(0, 2, 1, 3).reshape(E, P, KH * C))

    # Pre-blocked w2: w2b[e, hb, r, ib*128+c] = w2[e, ib*128+r, hb*128+c]
    w2b = np.ascontiguousarray(
        w2.reshape(E, MI, P, MH, P).transpose(0, 3, 2, 1, 4).reshape(E, MH, P, I)
    ).astype(wdt)
    w1c = w1.astype(wdt)
    w3c = w3.astype(wdt)

    nc = _build(C, PREC)

    in_maps = []
    for c in range(N_CORES):
        sl = slice(E_LOC * c, E_LOC * (c + 1))
        in_maps.append({
            "xT": np.ascontiguousarray(xT_tiled[sl]),
            "w1": np.ascontiguousarray(w1c[sl]),
            "w3": np.ascontiguousarray(w3c[sl]),
            "w2b": np.ascontiguousarray(w2b[sl]),
        })

    kwargs = {}
    if TRACE:
        kwargs.update(trace=True, trace_cores=[0])
        if os.environ.get("MOE_TMPDIR"):
            global _RUN_IDX
            _RUN_IDX += 1
            td = os.path.join(os.environ["MOE_TMPDIR"], f"r{_RUN_IDX}")
            os.makedirs(td, exist_ok=True)
            kwargs["tmpdir"] = td
    res = run_bass_kernel_spmd(nc, in_maps, core_ids=list(range(N_CORES)), **kwargs)
    LAST_RESULTS = res

    MH = H // P
    out = np.zeros((T, H), dtype=np.float64)
    for c in range(N_CORES):
        yTt = res.results[c]["yT"]  # [E_LOC, P, MH*C] fp32, tiled
        for j in range(E_LOC):
            e = E_LOC * c + j
            n = len(tok[e])
            if n:
                # un-tile: [P, MH, C] -> [MH, P, C] -> [H, C]
                y_full = yTt[j].reshape(P, MH, C).transpose(1, 0, 2).reshape(H, C)
                y = y_full[:, :n].T.astype(np.float64)  # [n, H]
                out[tok[e]] += wgt[e][:, None] * y
    return out.astype(np.float32)
